# revision 4
# baseline (speedup 1.0000x reference)
"""Trainium2 Bass kernel for DeformationTrackerBiFlowModel — G=7, fused-DVE out.

Reference math (per batch element b, per step t):
    x_t   = [prev_out (2), fin_t (3)]            (5,)
    h_t   = tanh(x_t @ W_rnn + b_rnn)            (12,)   (U_rnn is inert)
    out_t = [cp0 (2), h_t (12)] @ W_out + b_out  (2,)
    prev_out_{t+1} = out_t;  prev_out_0 = cp0

Folded recurrence: pre_t = h_{t-1} @ Wh + fin_t @ W1f + 1*r + cp0 @ E with
Wh = Wo2 @ W1p; h_t = tanh(pre_t); out_t = cvec + h_t @ Wo2 (cvec host-added).

G=7 trajectories block-diagonal per column, K=120, M=110 (pre 0:84 | pad |
oraw 96:110 where oraw_t = h_{t-1} @ Wo2 = out_{t-1} - cvec).  One matmul +
one tanh per chain-step; the three chains' psum lives in ONE 3-bank psum
tile (chain ch at column offset 512*ch) so a single DVE copy per step
stages all chains' oraw rows (free size 3*COLS amortizes DVE overhead and
the copy is a sink — off the critical path).  cvec is added on the host.

Per step: PE 3 matmuls (one weight tile, no swaps), ACT 3x tanh[84,391],
DVE 1x copy[14, 3x391], DMA: fin 21-desc/4-step group (contiguous staging),
out 14-desc/4-step group.  Batch 65536 over 8 cores; G*C*COLS = 8211.
"""

import os
from contextlib import ExitStack

import numpy as np

import concourse.mybir as mybir
import concourse.tile as tile
from concourse import bacc
from concourse.bass_utils import run_bass_kernel_spmd

B, T = 65536, 100
D_CP, D_FIN, HID = 2, 3, 12
NCORES = 8
BC = B // NCORES              # 8192 per core
G = 7                         # trajectories packed per matmul (block-diag)
C = 3                         # independent column chains
COLS = 391                    # batch columns per chain
BP = G * C * COLS             # 8211 padded batch per core
NH = HID * G                  # 84: h rows (rhs) / pre rows (psum)
NFIN = D_FIN * G              # 21 fin rows
NCONST = 1 + D_CP * G         # 15: ones + cp0 rows
KTOT = NH + NFIN + NCONST     # 120
MOUT = D_CP * G               # 14 oraw rows
MPAD = 96                     # oraw region starts at psum partition 96
MTOT = MPAD + MOUT            # 110
PW = 512                      # psum column stride per chain (bank aligned)
NFG = (T + 3) // 4 + 1        # 26 fin groups (steps 4g..4g+3, zero padded)

F32 = mybir.dt.float32

_MM_CHOICES = {"bf16": mybir.dt.bfloat16, "f32r": mybir.dt.float32r, "f32": F32}
MM_DTYPE = _MM_CHOICES[os.environ.get("DTB_MM", "bf16")]
MM_NP = mybir.dt.np(MM_DTYPE)

LAST_RESULTS = None  # test.py introspects profiling info from here


def build_program(t_steps=T, g=G, c=C, cols=COLS, mm_dtype=None):
    if mm_dtype is None:
        mm_dtype = MM_DTYPE
    XDT = mm_dtype
    nh, nfin, nconst = HID * g, D_FIN * g, 1 + D_CP * g
    ktot = nh + nfin + nconst
    mout = D_CP * g
    mpad, mtot = MPAD, MPAD + D_CP * g
    nfg = (t_steps + 3) // 4 + 1
    nog = t_steps // 4
    nc = bacc.Bacc(target_bir_lowering=False)

    fin = nc.dram_tensor("fin", [c, nfin, nfg, 4 * cols], XDT, kind="ExternalInput")
    xc = nc.dram_tensor("xc", [c, nconst, 8 * cols], XDT, kind="ExternalInput")
    w = nc.dram_tensor("w", [ktot, mtot], XDT, kind="ExternalInput")
    w0 = nc.dram_tensor("w0", [ktot, mtot], XDT, kind="ExternalInput")
    out = nc.dram_tensor("out", [nog, mout, 4 * c * cols], XDT, kind="ExternalOutput")

    tanh = mybir.ActivationFunctionType.Tanh

    with tile.TileContext(nc) as tc, ExitStack() as ctx:
        const = ctx.enter_context(tc.tile_pool(name="const", bufs=1))
        xpool = ctx.enter_context(tc.tile_pool(name="xpool", bufs=1))
        opool = ctx.enter_context(tc.tile_pool(name="opool", bufs=3))
        psum = ctx.enter_context(tc.tile_pool(name="psum", bufs=2, space="PSUM"))

        # Weight loads split into partition chunks across two queues so the
        # ~120 per-partition descriptors stream in parallel.
        w0s = const.tile([ktot, mtot], XDT, name="w0s")
        ws = const.tile([ktot, mtot], XDT, name="ws")
        for i in range(4):
            r0, r1 = 30 * i, 30 * (i + 1)
            (nc.sync if i % 2 == 0 else nc.gpsimd).dma_start(
                out=w0s[r0:r1, :], in_=w0[r0:r1, :]
            )
        for i in range(4):
            r0, r1 = 30 * i, 30 * (i + 1)
            (nc.gpsimd if i % 2 == 0 else nc.sync).dma_start(
                out=ws[r0:r1, :], in_=w[r0:r1, :]
            )

        # One persistent rhs tile per chain, 8 column-blocks (block = t % 8).
        # Rows: h 0:84 (ACT) | fin 84:105 (DMA) | ones+cp0 105:120 (once).
        xtiles = []
        for ch in range(c):
            xt = xpool.tile([ktot, 8 * cols], XDT, tag=f"x{ch}", name=f"x_{ch}")
            nc.vector.memset(xt[0:nh, :], 0)
            (nc.sync if ch % 2 == 0 else nc.gpsimd).dma_start(
                out=xt[nh + nfin :, :], in_=xc[ch]
            )
            for gg in range(2):  # fin groups 0,1 = steps 0..7
                nc.sync.dma_start(
                    out=xt[nh : nh + nfin, 4 * gg * cols : (4 * gg + 4) * cols],
                    in_=fin[ch, :, gg, :],
                )
            xtiles.append(xt)

        ost = None
        for t in range(t_steps + 1):
            p1 = psum.tile([mtot, c * PW], F32, tag="p", name=f"p_{t}")
            for ch in range(c):
                xt = xtiles[ch]
                blk = t % 8
                pv = p1[:, ch * PW : ch * PW + cols]
                nc.tensor.matmul(
                    pv, w0s if t == 0 else ws,
                    xt[:, blk * cols : (blk + 1) * cols], start=True, stop=True,
                )
                if t < t_steps:
                    nb = (t + 1) % 8
                    nc.scalar.activation(
                        xt[0:nh, nb * cols : (nb + 1) * cols], pv[0:nh, :], tanh
                    )
            if t > 0:
                # Stage oraw_{t-1} for all chains with one DVE copy.
                ob = (t - 1) % 4
                if ob == 0:
                    ost = opool.tile([mout, 4 * c * cols], XDT, tag="o",
                                     name=f"o_{t}")
                src = p1[mpad:mtot, :].rearrange("r (b q) -> r b q", b=c)[
                    :, :, 0:cols
                ]
                nc.vector.tensor_copy(
                    out=ost[:, ob * c * cols : (ob + 1) * c * cols].rearrange(
                        "r (b q) -> r b q", b=c
                    ),
                    in_=src,
                )
                if ob == 3:
                    nc.gpsimd.dma_start(out=out[(t - 4) // 4], in_=ost)
            # Prefetch fin group g = steps 4g..4g+3, ~5 steps ahead.
            if t % 4 == 3:
                gg = (t + 5) // 4
                if gg < nfg:
                    b0 = (4 * gg) % 8
                    for ch in range(c):
                        nc.sync.dma_start(
                            out=xtiles[ch][
                                nh : nh + nfin, b0 * cols : (b0 + 4) * cols
                            ],
                            in_=fin[ch, :, gg, :],
                        )
    nc.compile()
    return nc


def build_packed_weights(W_rnn, W_out, b_rnn, b_out, g=G):
    W_rnn = np.asarray(W_rnn, np.float32)
    W_out = np.asarray(W_out, np.float32)
    b_rnn = np.asarray(b_rnn, np.float32)
    b_out = np.asarray(b_out, np.float32)
    W1p, W1f = W_rnn[:D_CP], W_rnn[D_CP:]
    Wo1, Wo2 = W_out[:D_CP], W_out[D_CP:]
    nh, nfin = HID * g, D_FIN * g
    ktot = nh + nfin + 1 + D_CP * g
    mpad, mtot = MPAD, MPAD + D_CP * g
    ones_row = nh + nfin
    cp0_base = ones_row + 1

    E = Wo1 @ W1p
    r = b_rnn + b_out @ W1p
    Wh = Wo2 @ W1p
    w = np.zeros((ktot, mtot), np.float32)
    w0 = np.zeros((ktot, mtot), np.float32)
    for i in range(g):
        hsl = slice(HID * i, HID * (i + 1))
        osl = slice(mpad + D_CP * i, mpad + D_CP * (i + 1))
        w[hsl, hsl] = Wh
        w[hsl, osl] = Wo2
        w0[hsl, osl] = Wo2
        fsl = slice(nh + D_FIN * i, nh + D_FIN * (i + 1))
        w[fsl, hsl] = W1f
        w0[fsl, hsl] = W1f
        w[ones_row, hsl] = r
        w0[ones_row, hsl] = b_rnn
        csl = slice(cp0_base + D_CP * i, cp0_base + D_CP * (i + 1))
        w[csl, hsl] = E
        w0[csl, hsl] = W1p
    return w, w0


def stage_inputs(cp0, fin, g=G, c=C, cols=COLS, t_steps=T):
    """Batch-major -> feature-major device layouts (b = ch*(g*cols)+gi*cols+j)."""
    bp = g * c * cols
    bc = cp0.shape[0]
    nfg = (t_steps + 3) // 4 + 1
    F = np.zeros((bp, 4 * nfg, D_FIN), np.float32)
    F[:bc, :t_steps] = fin
    cp0_p = np.zeros((bp, D_CP), np.float32)
    cp0_p[:bc] = cp0
    fin_d = np.ascontiguousarray(
        F.reshape(c, g, cols, nfg, 4, D_FIN).transpose(0, 1, 5, 3, 4, 2)
    ).reshape(c, D_FIN * g, nfg, 4 * cols)
    xc_d = np.ones((c, 1 + D_CP * g, cols), np.float32)
    xc_d[:, 1:, :] = (
        cp0_p.reshape(c, g, cols, D_CP).transpose(0, 1, 3, 2).reshape(c, D_CP * g, cols)
    )
    xc_d = np.tile(xc_d, (1, 1, 8))
    return fin_d, xc_d


def unstage_output(out_d, cvec_p, bc, g=G, c=C, cols=COLS, t_steps=T):
    """out_d [T/4, 14, 4*c*cols] bf16 oraw -> out [bc, T, 2] f32."""
    bp = g * c * cols
    nog = t_steps // 4
    o = out_d.reshape(nog, g, D_CP, 4, c, cols).transpose(4, 1, 5, 0, 3, 2)
    o = np.ascontiguousarray(o, np.float32).reshape(bp, t_steps, D_CP)
    o += cvec_p[:, None, :]
    return o[:bc]


def kernel(control_point_input, finger_input, W_rnn, U_rnn, b_rnn, W_out, b_out):
    global LAST_RESULTS
    cp = np.asarray(control_point_input, np.float32)
    fin = np.asarray(finger_input, np.float32)
    W_rnn = np.asarray(W_rnn, np.float32)
    b_rnn = np.asarray(b_rnn, np.float32)
    W_out = np.asarray(W_out, np.float32)
    b_out = np.asarray(b_out, np.float32)

    cp0 = cp[:, 0, :]
    cvec = cp0 @ W_out[:D_CP] + b_out
    w, w0 = build_packed_weights(W_rnn, W_out, b_rnn, b_out)
    w, w0 = (x.astype(MM_NP) for x in (w, w0))

    nc = build_program()
    in_maps = []
    for m in range(NCORES):
        sl = slice(m * BC, (m + 1) * BC)
        fin_d, xc_d = stage_inputs(cp0[sl], fin[sl])
        in_maps.append(
            {"fin": fin_d.astype(MM_NP, copy=False),
             "xc": xc_d.astype(MM_NP, copy=False), "w": w, "w0": w0}
        )

    trace = bool(os.environ.get("DTB_TRACE"))
    res = run_bass_kernel_spmd(
        nc, in_maps, core_ids=list(range(NCORES)), trace=trace
    )
    LAST_RESULTS = res

    outs = []
    for m in range(NCORES):
        sl = slice(m * BC, (m + 1) * BC)
        cvec_p = np.zeros((BP, D_CP), np.float32)
        cvec_p[:BC] = cvec[sl]
        outs.append(
            unstage_output(np.asarray(res.results[m]["out"]), cvec_p, BC)
        )
    return np.concatenate(outs, axis=0)


# revision 6
# speedup vs baseline: 1.5007x; 1.5007x over previous
"""Trainium2 Bass kernel for DeformationTrackerBiFlowModel — G=7, fused-DVE out.

Reference math (per batch element b, per step t):
    x_t   = [prev_out (2), fin_t (3)]            (5,)
    h_t   = tanh(x_t @ W_rnn + b_rnn)            (12,)   (U_rnn is inert)
    out_t = [cp0 (2), h_t (12)] @ W_out + b_out  (2,)
    prev_out_{t+1} = out_t;  prev_out_0 = cp0

Folded recurrence: pre_t = h_{t-1} @ Wh + fin_t @ W1f + 1*r + cp0 @ E with
Wh = Wo2 @ W1p; h_t = tanh(pre_t); out_t = cvec + h_t @ Wo2 (cvec host-added).

G=7 trajectories block-diagonal per column, K=120, M=110 (pre 0:84 | pad |
oraw 96:110 where oraw_t = h_{t-1} @ Wo2 = out_{t-1} - cvec).  One matmul +
one tanh per chain-step; the three chains' psum lives in ONE 3-bank psum
tile (chain ch at column offset 512*ch) so a single DVE copy per step
stages all chains' oraw rows (free size 3*COLS amortizes DVE overhead and
the copy is a sink — off the critical path).  cvec is added on the host.

Per step: PE 3 matmuls (one weight tile, no swaps), ACT 3x tanh[84,391],
DVE 1x copy[14, 3x391], DMA: fin 21-desc/4-step group (contiguous staging),
out 14-desc/4-step group.  Batch 65536 over 8 cores; G*C*COLS = 8211.
"""

import os
from contextlib import ExitStack

import numpy as np

import concourse.mybir as mybir
import concourse.tile as tile
from concourse import bacc
from concourse.bass_utils import run_bass_kernel_spmd

B, T = 65536, 100
D_CP, D_FIN, HID = 2, 3, 12
NCORES = 8
BC = B // NCORES              # 8192 per core
G = 7                         # trajectories packed per matmul (block-diag)
C = 3                         # independent column chains
COLS = 391                    # batch columns per chain
BP = G * C * COLS             # 8211 padded batch per core
NH = HID * G                  # 84: h rows (rhs) / pre rows (psum)
NFIN = D_FIN * G              # 21 fin rows
NCONST = 1 + D_CP * G         # 15: ones + cp0 rows
KTOT = NH + NFIN + NCONST     # 120
MOUT = D_CP * G               # 14 oraw rows
MPAD = 96                     # oraw region starts at psum partition 96
MTOT = MPAD + MOUT            # 110
PW = 512                      # psum column stride per chain (bank aligned)
NFG = (T + 3) // 4 + 1        # 26 fin groups (steps 4g..4g+3, zero padded)

F32 = mybir.dt.float32

_MM_CHOICES = {"bf16": mybir.dt.bfloat16, "f32r": mybir.dt.float32r, "f32": F32}
MM_DTYPE = _MM_CHOICES[os.environ.get("DTB_MM", "bf16")]
MM_NP = mybir.dt.np(MM_DTYPE)

LAST_RESULTS = None  # test.py introspects profiling info from here


def build_program(t_steps=T, g=G, c=C, cols=COLS, mm_dtype=None):
    if mm_dtype is None:
        mm_dtype = MM_DTYPE
    XDT = mm_dtype
    nh, nfin, nconst = HID * g, D_FIN * g, 1 + D_CP * g
    ktot = nh + nfin + nconst
    mout = D_CP * g
    mpad, mtot = MPAD, MPAD + D_CP * g
    nfg = (t_steps + 3) // 4 + 1
    nog = t_steps // 4
    nc = bacc.Bacc(target_bir_lowering=False)

    fin = nc.dram_tensor("fin", [c, nfin, nfg, 4 * cols], XDT, kind="ExternalInput")
    xc = nc.dram_tensor("xc", [c, nconst, 8 * cols], XDT, kind="ExternalInput")
    w = nc.dram_tensor("w", [ktot, mtot], XDT, kind="ExternalInput")
    w0 = nc.dram_tensor("w0", [ktot, mtot], XDT, kind="ExternalInput")
    out = nc.dram_tensor("out", [nog, mout, 4 * c * cols], XDT, kind="ExternalOutput")

    tanh = mybir.ActivationFunctionType.Tanh

    with tile.TileContext(nc) as tc, ExitStack() as ctx:
        const = ctx.enter_context(tc.tile_pool(name="const", bufs=1))
        xpool = ctx.enter_context(tc.tile_pool(name="xpool", bufs=1))
        opool = ctx.enter_context(tc.tile_pool(name="opool", bufs=3))
        psum = ctx.enter_context(tc.tile_pool(name="psum", bufs=1, space="PSUM"))

        # Persistent 8-bank PSUM arena, windows managed by hand: (chain ch,
        # parity q) lives at column 512*(ch + c*q).  Region-level dependency
        # tracking then keeps the chains independent (a shared per-step pool
        # tile would couple every matmul to ALL previous consumers).
        arena = psum.tile([mtot, 8 * PW], F32, name="parena")

        # Weight loads split into partition chunks across two queues so the
        # ~120 per-partition descriptors stream in parallel.
        w0s = const.tile([ktot, mtot], XDT, name="w0s")
        ws = const.tile([ktot, mtot], XDT, name="ws")
        for i in range(4):
            r0, r1 = 30 * i, 30 * (i + 1)
            (nc.sync if i % 2 == 0 else nc.gpsimd).dma_start(
                out=w0s[r0:r1, :], in_=w0[r0:r1, :]
            )
        for i in range(4):
            r0, r1 = 30 * i, 30 * (i + 1)
            (nc.gpsimd if i % 2 == 0 else nc.sync).dma_start(
                out=ws[r0:r1, :], in_=w[r0:r1, :]
            )

        # One persistent rhs tile per chain, 8 column-blocks (block = t % 8).
        # Rows: h 0:84 (ACT) | fin 84:105 (DMA) | ones+cp0 105:120 (once).
        xtiles = []
        for ch in range(c):
            xt = xpool.tile([ktot, 8 * cols], XDT, tag=f"x{ch}", name=f"x_{ch}")
            nc.vector.memset(xt[0:nh, :], 0)
            (nc.sync if ch % 2 == 0 else nc.gpsimd).dma_start(
                out=xt[nh + nfin :, :], in_=xc[ch]
            )
            for gg in range(2):  # fin groups 0,1 = steps 0..7
                nc.sync.dma_start(
                    out=xt[nh : nh + nfin, 4 * gg * cols : (4 * gg + 4) * cols],
                    in_=fin[ch, :, gg, :],
                )
            xtiles.append(xt)

        ost = None
        for t in range(t_steps + 1):
            q = t % 2
            p1 = arena[:, q * c * PW : (q + 1) * c * PW]
            for ch in range(c):
                xt = xtiles[ch]
                blk = t % 8
                pv = p1[:, ch * PW : ch * PW + cols]
                nc.tensor.matmul(
                    pv, w0s if t == 0 else ws,
                    xt[:, blk * cols : (blk + 1) * cols], start=True, stop=True,
                )
                if t < t_steps:
                    nb = (t + 1) % 8
                    nc.scalar.activation(
                        xt[0:nh, nb * cols : (nb + 1) * cols], pv[0:nh, :], tanh
                    )
            if t > 0:
                # Stage oraw_{t-1} for all chains with one DVE copy.
                ob = (t - 1) % 4
                if ob == 0:
                    ost = opool.tile([mout, 4 * c * cols], XDT, tag="o",
                                     name=f"o_{t}")
                src = p1[mpad:mtot, :].rearrange("r (b q) -> r b q", b=c)[
                    :, :, 0:cols
                ]
                nc.vector.tensor_copy(
                    out=ost[:, ob * c * cols : (ob + 1) * c * cols].rearrange(
                        "r (b q) -> r b q", b=c
                    ),
                    in_=src,
                )
                if ob == 3:
                    nc.gpsimd.dma_start(out=out[(t - 4) // 4], in_=ost)
            # Prefetch fin group g = steps 4g..4g+3, ~5 steps ahead.
            if t % 4 == 3:
                gg = (t + 5) // 4
                if gg < nfg:
                    b0 = (4 * gg) % 8
                    for ch in range(c):
                        nc.sync.dma_start(
                            out=xtiles[ch][
                                nh : nh + nfin, b0 * cols : (b0 + 4) * cols
                            ],
                            in_=fin[ch, :, gg, :],
                        )
    nc.compile()
    return nc


def build_packed_weights(W_rnn, W_out, b_rnn, b_out, g=G):
    W_rnn = np.asarray(W_rnn, np.float32)
    W_out = np.asarray(W_out, np.float32)
    b_rnn = np.asarray(b_rnn, np.float32)
    b_out = np.asarray(b_out, np.float32)
    W1p, W1f = W_rnn[:D_CP], W_rnn[D_CP:]
    Wo1, Wo2 = W_out[:D_CP], W_out[D_CP:]
    nh, nfin = HID * g, D_FIN * g
    ktot = nh + nfin + 1 + D_CP * g
    mpad, mtot = MPAD, MPAD + D_CP * g
    ones_row = nh + nfin
    cp0_base = ones_row + 1

    E = Wo1 @ W1p
    r = b_rnn + b_out @ W1p
    Wh = Wo2 @ W1p
    w = np.zeros((ktot, mtot), np.float32)
    w0 = np.zeros((ktot, mtot), np.float32)
    for i in range(g):
        hsl = slice(HID * i, HID * (i + 1))
        osl = slice(mpad + D_CP * i, mpad + D_CP * (i + 1))
        w[hsl, hsl] = Wh
        w[hsl, osl] = Wo2
        w0[hsl, osl] = Wo2
        fsl = slice(nh + D_FIN * i, nh + D_FIN * (i + 1))
        w[fsl, hsl] = W1f
        w0[fsl, hsl] = W1f
        w[ones_row, hsl] = r
        w0[ones_row, hsl] = b_rnn
        csl = slice(cp0_base + D_CP * i, cp0_base + D_CP * (i + 1))
        w[csl, hsl] = E
        w0[csl, hsl] = W1p
    return w, w0


def stage_inputs(cp0, fin, g=G, c=C, cols=COLS, t_steps=T):
    """Batch-major -> feature-major device layouts (b = ch*(g*cols)+gi*cols+j)."""
    bp = g * c * cols
    bc = cp0.shape[0]
    nfg = (t_steps + 3) // 4 + 1
    F = np.zeros((bp, 4 * nfg, D_FIN), np.float32)
    F[:bc, :t_steps] = fin
    cp0_p = np.zeros((bp, D_CP), np.float32)
    cp0_p[:bc] = cp0
    fin_d = np.ascontiguousarray(
        F.reshape(c, g, cols, nfg, 4, D_FIN).transpose(0, 1, 5, 3, 4, 2)
    ).reshape(c, D_FIN * g, nfg, 4 * cols)
    xc_d = np.ones((c, 1 + D_CP * g, cols), np.float32)
    xc_d[:, 1:, :] = (
        cp0_p.reshape(c, g, cols, D_CP).transpose(0, 1, 3, 2).reshape(c, D_CP * g, cols)
    )
    xc_d = np.tile(xc_d, (1, 1, 8))
    return fin_d, xc_d


def unstage_output(out_d, cvec_p, bc, g=G, c=C, cols=COLS, t_steps=T):
    """out_d [T/4, 14, 4*c*cols] bf16 oraw -> out [bc, T, 2] f32."""
    bp = g * c * cols
    nog = t_steps // 4
    o = out_d.reshape(nog, g, D_CP, 4, c, cols).transpose(4, 1, 5, 0, 3, 2)
    o = np.ascontiguousarray(o, np.float32).reshape(bp, t_steps, D_CP)
    o += cvec_p[:, None, :]
    return o[:bc]


def kernel(control_point_input, finger_input, W_rnn, U_rnn, b_rnn, W_out, b_out):
    global LAST_RESULTS
    cp = np.asarray(control_point_input, np.float32)
    fin = np.asarray(finger_input, np.float32)
    W_rnn = np.asarray(W_rnn, np.float32)
    b_rnn = np.asarray(b_rnn, np.float32)
    W_out = np.asarray(W_out, np.float32)
    b_out = np.asarray(b_out, np.float32)

    cp0 = cp[:, 0, :]
    cvec = cp0 @ W_out[:D_CP] + b_out
    w, w0 = build_packed_weights(W_rnn, W_out, b_rnn, b_out)
    w, w0 = (x.astype(MM_NP) for x in (w, w0))

    nc = build_program()
    in_maps = []
    for m in range(NCORES):
        sl = slice(m * BC, (m + 1) * BC)
        fin_d, xc_d = stage_inputs(cp0[sl], fin[sl])
        in_maps.append(
            {"fin": fin_d.astype(MM_NP, copy=False),
             "xc": xc_d.astype(MM_NP, copy=False), "w": w, "w0": w0}
        )

    trace = bool(os.environ.get("DTB_TRACE"))
    res = run_bass_kernel_spmd(
        nc, in_maps, core_ids=list(range(NCORES)), trace=trace
    )
    LAST_RESULTS = res

    outs = []
    for m in range(NCORES):
        sl = slice(m * BC, (m + 1) * BC)
        cvec_p = np.zeros((BP, D_CP), np.float32)
        cvec_p[:BC] = cvec[sl]
        outs.append(
            unstage_output(np.asarray(res.results[m]["out"]), cvec_p, BC)
        )
    return np.concatenate(outs, axis=0)
